# revision 20
# baseline (speedup 1.0000x reference)
import sys

sys.path.insert(0, "/opt/trn_rl_repo")
import os as _os
import numpy as np
import concourse.bass as bass
import concourse.tile as tile
from concourse import bacc, mybir
from concourse.bass_utils import run_bass_kernel_spmd

F32 = mybir.dt.float32
F32R = mybir.dt.float32r
BF16 = mybir.dt.bfloat16
AF = mybir.ActivationFunctionType

B, T, C = 64, 500, 256
E, H, D = 512, 8, 64
N_CORES = 8
BL = B // N_CORES  # batches per core

# matmul operand mode: bf16 (1 cyc/row on hw), f32r (~2 cyc/row on hw), f32 (4)
MM_MODE = _os.environ.get("MM_MODE", "bf16")
# ablation for profiling: comma-set of {stmm, exp, yt, z, norm, qkv, va, proj}
ABLATE = set(filter(None, _os.environ.get("ABLATE", "").split(",")))

TT = [128, 128, 128, 116]  # t/s tile sizes (500 = 3*128 + 116)

MM_DT = {"bf16": BF16, "f32r": F32R, "f32": F32}[MM_MODE]
# dram x/w tensors are host-converted to bf16 in bf16 mode; else f32 (+bitcast)
IN_DT = BF16 if MM_MODE == "bf16" else F32

# tensor specs for timeit_hw's loop-variant builder: (name, shape, dtype)
INPUT_SPECS = [
    ("xt", (BL, C, T), IN_DT),
    ("wat", (C, 3 * E), IN_DT),
    ("wpt", (E, E), IN_DT),
    ("bqk", (128, 8), F32),
    ("bvb", (128, E), F32),
    ("bpb", (128, E), F32),
]
OUTPUT_SPEC = (BL, T, E)


def _mm(ap):
    # reinterpret f32 as f32r for the tensor engine (no-op in bf16/f32 modes)
    return ap if ap.dtype == MM_DT or MM_DT == BF16 else ap.bitcast(MM_DT)


def build_nc():
    nc = bacc.Bacc("TRN2", target_bir_lowering=False)
    aps = {}
    for name, shape, dt in INPUT_SPECS:
        aps[name] = nc.dram_tensor(name, list(shape), dt, kind="ExternalInput")
    out = nc.dram_tensor("out", list(OUTPUT_SPEC), F32, kind="ExternalOutput")

    with tile.TileContext(nc) as tc:
        _build_body(nc, tc, out=out, **aps)
    nc.compile()
    return nc


def _build_body(nc, tc, xt, wat, wpt, bqk, bvb, bpb, out, pfx=""):
    from contextlib import ExitStack

    ctx = ExitStack()
    with ctx:
        cpool = ctx.enter_context(tc.tile_pool(name="consts", bufs=1))
        xpool = ctx.enter_context(tc.tile_pool(name="x", bufs=2))
        qkpool = ctx.enter_context(tc.tile_pool(name="qk", bufs=2))
        vpool = ctx.enter_context(tc.tile_pool(name="v", bufs=2))
        epool = ctx.enter_context(tc.tile_pool(name="est", bufs=4))
        ypool = ctx.enter_context(tc.tile_pool(name="yt", bufs=2))
        opool = ctx.enter_context(tc.tile_pool(name="os", bufs=2))
        zpool = ctx.enter_context(tc.tile_pool(name="zr", bufs=3))
        # PSUM pools: 8 banks total
        EXPCHUNK = _os.environ.get("EXPCHUNK", "1") == "1"
        ps_io = ctx.enter_context(tc.tile_pool(name="ps_io", bufs=2, space="PSUM"))
        ps_st = ctx.enter_context(
            tc.tile_pool(name="ps_st", bufs=2 if EXPCHUNK else 4, space="PSUM"))
        ps_yt = ctx.enter_context(tc.tile_pool(name="ps_yt", bufs=2, space="PSUM"))

        # ---- constants ----
        wa = cpool.tile([128, 2 * 3 * E], MM_DT, name=f"{pfx}wa")  # 2 c-ktiles x [128,1536]
        for k in range(2):
            nc.sync.dma_start(wa[:, k * 1536:(k + 1) * 1536], _mm(wat[k * 128:(k + 1) * 128, :]))
        wp = cpool.tile([128, 4 * E], MM_DT, name=f"{pfx}wp")  # 4 e-ktiles x [128,512]
        for k in range(4):
            nc.sync.dma_start(wp[:, k * E:(k + 1) * E], _mm(wpt[k * 128:(k + 1) * 128, :]))
        bqk_t = cpool.tile([128, 8], F32, name=f"{pfx}bqk_t")
        nc.sync.dma_start(bqk_t[:], bqk[:, :])
        bvb_t = cpool.tile([128, E], F32, name=f"{pfx}bvb_t")
        nc.sync.dma_start(bvb_t[:], bvb[:, :])
        bpb_t = cpool.tile([128, E], F32, name=f"{pfx}bpb_t")
        nc.sync.dma_start(bpb_t[:], bpb[:, :])
        NORM_DIV = _os.environ.get("NORM_DIV", "0") == "1"
        QKCOPY_DVE = _os.environ.get("QKCOPY_DVE", "0") == "1"

        def do_proj_mt(yt, b, osb, mt):
            # out[t,f] = yT^T @ wpT + bproj (one 128-row t-tile)
            tt = TT[mt]
            po = ps_io.tile([128, E], F32, name=f"{pfx}po{b}_{mt}", tag="ps_io")
            if "proj" not in ABLATE:
                for k in range(4):
                    nc.tensor.matmul(
                        po[0:tt, :],
                        _mm(yt[:, k * T + mt * 128:k * T + mt * 128 + tt]),
                        _mm(wp[:, k * E:(k + 1) * E]),
                        start=(k == 0), stop=(k == 3),
                    )
                nc.vector.tensor_add(osb[0:tt, mt * E:(mt + 1) * E], po[0:tt, :], bpb_t[0:tt, :])
                nc.sync.dma_start(out[b, mt * 128:mt * 128 + tt, :], osb[0:tt, mt * E:(mt + 1) * E])

        def do_proj(yt, b):
            osb = opool.tile([128, 4 * E], F32, name=f"{pfx}osb{b}", tag="osb")
            for mt in range(4):
                do_proj_mt(yt, b, osb, mt)

        def load_x(b):
            xtb = xpool.tile([128, 2 * T], MM_DT, name=f"{pfx}xtb{b}", tag="xtb")
            for k in range(2):
                nc.sync.dma_start(xtb[:, k * T:(k + 1) * T], _mm(xt[b, k * 128:(k + 1) * 128, :]))
            return xtb

        # v_aug layout per s-chunk c: [st, 8 heads x 128] where per head
        # cols 0:64 = v+bias, cols 64:128 = ones (z-broadcast rows in yt mm).
        # ones blocks written once here; manual 2-deep rotation across batches.
        va_bufs = []
        for i in range(2):
            vab = vpool.tile([128, 4 * 1024], MM_DT, name=f"{pfx}vab{i}")
            va_bufs.append(vab)
            for c in range(4):
                va4 = vab[:, c * 1024:(c + 1) * 1024].rearrange("p (h m) -> p h m", h=H)
                nc.vector.memset(va4[:, :, 64:128], 1.0)

        xtb_next = load_x(0)
        prev = None  # (yt, b) awaiting projection
        for b in range(BL):
            xtb = xtb_next

            # ---- qkT(b): [1024(e), 500(t)] as [128, 8*500], e-tile j in cols ----
            qk = qkpool.tile([128, 8 * T], MM_DT, name=f"{pfx}qk{b}", tag="qk")
            for m in range(8):
                pq = ps_io.tile([128, T], F32, name=f"{pfx}pq{b}_{m}", tag="ps_io")
                if "qkv" in ABLATE:
                    continue
                for k in range(2):
                    nc.tensor.matmul(
                        pq[:],
                        _mm(wa[:, k * 1536 + m * 128:k * 1536 + (m + 1) * 128]),
                        _mm(xtb[:, k * T:(k + 1) * T]),
                        start=(k == 0), stop=(k == 1),
                    )
                if "qkcopy" in ABLATE:
                    continue
                # add per-partition bias (b_attn for q/k) while copying to SBUF
                if QKCOPY_DVE:
                    nc.vector.tensor_scalar_add(qk[:, m * T:(m + 1) * T], pq[:],
                                                bqk_t[:, m:m + 1])
                else:
                    nc.scalar.activation(qk[:, m * T:(m + 1) * T], pq[:], AF.Identity,
                                         bias=bqk_t[:, m:m + 1])

            # ---- v(b): [500(t), 512(e)] + bias, into v_aug [128, 4*1024] ----
            va = va_bufs[b % 2]
            for mt in range(4):
                tt = TT[mt]
                pv = ps_io.tile([128, E], F32, name=f"{pfx}pv{b}_{mt}", tag="ps_io")
                if "va" in ABLATE:
                    continue
                for k in range(2):
                    nc.tensor.matmul(
                        pv[0:tt, :],
                        _mm(xtb[:, k * T + mt * 128:k * T + mt * 128 + tt]),
                        _mm(wa[:, k * 1536 + 1024:k * 1536 + 1536]),
                        start=(k == 0), stop=(k == 1),
                    )
                va3 = va[:, mt * 1024:(mt + 1) * 1024].rearrange("p (h m) -> p h m", h=H)
                nc.vector.tensor_add(
                    va3[0:tt, :, 0:64],
                    pv[0:tt, :].rearrange("p (h m) -> p h m", h=H),
                    bvb_t[0:tt, :].rearrange("p (h m) -> p h m", h=H),
                )

            if b + 1 < BL:
                xtb_next = load_x(b + 1)

            # ---- attention heads, z-chain pipelined 2 deep ----
            # (projection of batch b-1 is interleaved into hp=0 fillers)
            est = {}
            pyt = {}
            zr = {}
            zbs = {}

            def do_st_half(h0, h1, sp, pst):
                # ST[s,t] = k_h @ q_h^T for s-pair sp; exp(ST/8) -> est[h]
                for h in (h0, h1):
                    jq, oq = h // 2, (h % 2) * 64
                    jk, ok = 4 + h // 2, (h % 2) * 64
                    # [128, 2*512]: each 500-col matmul half bank-aligned
                    pst[h] = ps_st.tile([128, 1024], F32,
                                        name=f"{pfx}pst{b}_{h}_{sp}", tag="ps_st")
                    for si in range(2):
                        if "stmm" in ABLATE:
                            continue
                        s = 2 * sp + si
                        st = TT[s]
                        nc.tensor.matmul(
                            pst[h][0:st, si * 512:si * 512 + T],
                            _mm(qk[ok:ok + 64, jk * T + s * 128:jk * T + s * 128 + st]),
                            _mm(qk[oq:oq + 64, jq * T:(jq + 1) * T]),
                            start=True, stop=True,
                        )
                for h in (h0, h1):
                    if "exp" in ABLATE:
                        continue
                    pst3 = pst[h][:].rearrange("p (c m) -> p c m", c=2)
                    est3 = est[h][:, 2 * sp * T:(2 * sp + 2) * T].rearrange(
                        "p (c m) -> p c m", c=2)
                    nc.scalar.activation(est3[:, :, 0:T], pst3[:, :, 0:T],
                                         AF.Exp, scale=0.125)

            def do_yt(h):
                # yT[d,t] rows 0:64 (+ z bcast in rows 64:128) = [v_h|ones64]^T @ expST
                p = ps_yt.tile([128, T], F32, name=f"{pfx}pyt{b}_{h}", tag="ps_yt")
                pyt[h] = p
                e = est[h]
                if "yt" not in ABLATE:
                    for s in range(4):
                        st = TT[s]
                        nc.tensor.matmul(
                            p[:],
                            _mm(va[0:st, s * 1024 + 128 * h:s * 1024 + 128 * h + 128]),
                            _mm(e[0:st, s * T:(s + 1) * T]),
                            start=(s == 0), stop=(s == 3),
                        )
                if not NORM_DIV:
                    zs = zpool.tile([64, T], F32, name=f"{pfx}zbs{b}_{h}", tag="zbs")
                    zbs[h] = zs
                    if "z" not in ABLATE:
                        nc.vector.reciprocal(zs[:], p[64:128, :])

            def do_norm(h, yt):
                if "norm" in ABLATE:
                    return
                j, o = h // 2, (h % 2) * 64
                if NORM_DIV:
                    nc.vector.tensor_tensor(
                        yt[o:o + 64, j * T:(j + 1) * T], pyt[h][0:64, :],
                        pyt[h][64:128, :], mybir.AluOpType.divide,
                    )
                else:
                    nc.vector.tensor_mul(
                        yt[o:o + 64, j * T:(j + 1) * T], pyt[h][0:64, :], zbs[h][:]
                    )

            yt = ypool.tile([128, 4 * T], MM_DT, name=f"{pfx}yt{b}", tag="yt")
            osb_prev = None
            if prev is not None:
                osb_prev = opool.tile([128, 4 * E], F32,
                                      name=f"{pfx}osb{prev[1]}", tag="osb")
            for hp in range(H // 2):
                for h in (2 * hp, 2 * hp + 1):
                    est[h] = epool.tile([128, 4 * T], MM_DT,
                                        name=f"{pfx}est{b}_{h}", tag="est")
                pst = {}
                do_st_half(2 * hp, 2 * hp + 1, 0, pst)
                # filler A: PE/DVE work that is ready now, absorbing the
                # st(sp1)-waits-exp(sp0) PSUM-rotation stall
                if hp >= 1:
                    do_yt(2 * hp - 2)
                    do_norm(2 * hp - 2, yt)
                elif prev is not None:
                    do_proj_mt(prev[0], prev[1], osb_prev, 0)
                    do_proj_mt(prev[0], prev[1], osb_prev, 1)
                do_st_half(2 * hp, 2 * hp + 1, 1, pst)
                if hp >= 1:
                    do_yt(2 * hp - 1)
                    do_norm(2 * hp - 1, yt)
                elif prev is not None:
                    do_proj_mt(prev[0], prev[1], osb_prev, 2)
                    do_proj_mt(prev[0], prev[1], osb_prev, 3)
            for h in (H - 2, H - 1):
                do_yt(h)
                do_norm(h, yt)

            prev = (yt, b)
        do_proj(*prev)


_NC = None


def _get_nc():
    global _NC
    if _NC is None:
        _NC = build_nc()
    return _NC


def _np_bf16():
    import ml_dtypes
    return np.dtype(ml_dtypes.bfloat16)


def prep_inputs(x, w_attn, b_attn, w_proj, b_proj):
    x = np.asarray(x, np.float32)
    w_attn = np.asarray(w_attn, np.float32)
    b_attn = np.asarray(b_attn, np.float32)
    w_proj = np.asarray(w_proj, np.float32)
    b_proj = np.asarray(b_proj, np.float32)

    xt_all = np.ascontiguousarray(x.transpose(0, 2, 1))  # [B, C, T]
    wat = np.ascontiguousarray(w_attn.T)  # [C, 1536]
    wpt = np.ascontiguousarray(w_proj.T)  # [E, E]
    if MM_MODE == "bf16":
        bf = _np_bf16()
        xt_all = xt_all.astype(bf)
        wat = wat.astype(bf)
        wpt = wpt.astype(bf)
    bqk = np.ascontiguousarray(b_attn[:1024].reshape(8, 128).T)  # [128, 8]
    bvb = np.ascontiguousarray(np.tile(b_attn[1024:1536][None, :], (128, 1)))
    bpb = np.ascontiguousarray(np.tile(b_proj[None, :], (128, 1)))

    in_maps = []
    for c in range(N_CORES):
        in_maps.append({
            "xt": np.ascontiguousarray(xt_all[c * BL:(c + 1) * BL]),
            "wat": wat, "wpt": wpt, "bqk": bqk, "bvb": bvb, "bpb": bpb,
        })
    return in_maps


def kernel(x, w_attn, b_attn, w_proj, b_proj):
    nc = _get_nc()
    in_maps = prep_inputs(x, w_attn, b_attn, w_proj, b_proj)
    res = run_bass_kernel_spmd(nc, in_maps, core_ids=list(range(N_CORES)))
    out = np.concatenate([res.results[c]["out"] for c in range(N_CORES)], axis=0)
    return out.astype(np.float32)


# revision 22
# speedup vs baseline: 1.0472x; 1.0472x over previous
import sys

sys.path.insert(0, "/opt/trn_rl_repo")
import os as _os
import numpy as np
import concourse.bass as bass
import concourse.tile as tile
from concourse import bacc, mybir
from concourse.bass_utils import run_bass_kernel_spmd

F32 = mybir.dt.float32
F32R = mybir.dt.float32r
BF16 = mybir.dt.bfloat16
AF = mybir.ActivationFunctionType

B, T, C = 64, 500, 256
E, H, D = 512, 8, 64
N_CORES = 8
BL = B // N_CORES  # batches per core

# matmul operand mode: bf16 (1 cyc/row on hw), f32r (~2 cyc/row on hw), f32 (4)
MM_MODE = _os.environ.get("MM_MODE", "bf16")
# ablation for profiling: comma-set of {stmm, exp, yt, z, norm, qkv, va, proj}
ABLATE = set(filter(None, _os.environ.get("ABLATE", "").split(",")))

TT = [128, 128, 128, 116]  # t/s tile sizes (500 = 3*128 + 116)

MM_DT = {"bf16": BF16, "f32r": F32R, "f32": F32}[MM_MODE]
# dram x/w tensors are host-converted to bf16 in bf16 mode; else f32 (+bitcast)
IN_DT = BF16 if MM_MODE == "bf16" else F32

# tensor specs for timeit_hw's loop-variant builder: (name, shape, dtype)
INPUT_SPECS = [
    ("xt", (BL, C, T), IN_DT),
    ("wat", (C, 3 * E), IN_DT),
    ("wpt", (E, E), IN_DT),
    ("bqk", (128, 8), F32),
    ("bvb", (128, E), F32),
    ("bpb", (128, E), F32),
]
OUTPUT_SPEC = (BL, T, E)


def _mm(ap):
    # reinterpret f32 as f32r for the tensor engine (no-op in bf16/f32 modes)
    return ap if ap.dtype == MM_DT or MM_DT == BF16 else ap.bitcast(MM_DT)


def build_nc():
    nc = bacc.Bacc("TRN2", target_bir_lowering=False)
    aps = {}
    for name, shape, dt in INPUT_SPECS:
        aps[name] = nc.dram_tensor(name, list(shape), dt, kind="ExternalInput")
    out = nc.dram_tensor("out", list(OUTPUT_SPEC), F32, kind="ExternalOutput")

    with tile.TileContext(nc) as tc:
        _build_body(nc, tc, out=out, **aps)
    nc.compile()
    return nc


def _build_body(nc, tc, xt, wat, wpt, bqk, bvb, bpb, out, pfx=""):
    from contextlib import ExitStack

    ctx = ExitStack()
    with ctx:
        cpool = ctx.enter_context(tc.tile_pool(name="consts", bufs=1))
        xpool = ctx.enter_context(tc.tile_pool(name="x", bufs=2))
        qkpool = ctx.enter_context(tc.tile_pool(name="qk", bufs=2))
        vpool = ctx.enter_context(tc.tile_pool(name="v", bufs=2))
        epool = ctx.enter_context(tc.tile_pool(name="est", bufs=4))
        ypool = ctx.enter_context(tc.tile_pool(name="yt", bufs=2))
        opool = ctx.enter_context(tc.tile_pool(name="os", bufs=2))
        zpool = ctx.enter_context(tc.tile_pool(name="zr", bufs=3))
        # PSUM pools: 8 banks total
        EXPCHUNK = _os.environ.get("EXPCHUNK", "1") == "1"
        ps_io = ctx.enter_context(tc.tile_pool(name="ps_io", bufs=2, space="PSUM"))
        ps_st = ctx.enter_context(
            tc.tile_pool(name="ps_st", bufs=2 if EXPCHUNK else 4, space="PSUM"))
        ps_yt = ctx.enter_context(tc.tile_pool(name="ps_yt", bufs=2, space="PSUM"))

        # ---- constants ----
        wa = cpool.tile([128, 2 * 3 * E], MM_DT, name=f"{pfx}wa")  # 2 c-ktiles x [128,1536]
        for k in range(2):
            nc.sync.dma_start(wa[:, k * 1536:(k + 1) * 1536], _mm(wat[k * 128:(k + 1) * 128, :]))
        wp = cpool.tile([128, 4 * E], MM_DT, name=f"{pfx}wp")  # 4 e-ktiles x [128,512]
        for k in range(4):
            nc.sync.dma_start(wp[:, k * E:(k + 1) * E], _mm(wpt[k * 128:(k + 1) * 128, :]))
        bqk_t = cpool.tile([128, 8], F32, name=f"{pfx}bqk_t")
        nc.sync.dma_start(bqk_t[:], bqk[:, :])
        bvb_t = cpool.tile([128, E], F32, name=f"{pfx}bvb_t")
        nc.sync.dma_start(bvb_t[:], bvb[:, :])
        bpb_t = cpool.tile([128, E], F32, name=f"{pfx}bpb_t")
        nc.sync.dma_start(bpb_t[:], bpb[:, :])
        NORM_DIV = _os.environ.get("NORM_DIV", "0") == "1"
        QKCOPY_DVE = _os.environ.get("QKCOPY_DVE", "0")

        def do_proj_mt(yt, b, osb, mt):
            # out[t,f] = yT^T @ wpT + bproj (one 128-row t-tile)
            tt = TT[mt]
            po = ps_io.tile([128, E], F32, name=f"{pfx}po{b}_{mt}", tag="ps_io")
            if "proj" not in ABLATE:
                for k in range(4):
                    nc.tensor.matmul(
                        po[0:tt, :],
                        _mm(yt[:, k * T + mt * 128:k * T + mt * 128 + tt]),
                        _mm(wp[:, k * E:(k + 1) * E]),
                        start=(k == 0), stop=(k == 3),
                    )
                nc.vector.tensor_add(osb[0:tt, mt * E:(mt + 1) * E], po[0:tt, :], bpb_t[0:tt, :])
                nc.sync.dma_start(out[b, mt * 128:mt * 128 + tt, :], osb[0:tt, mt * E:(mt + 1) * E])

        def do_proj(yt, b):
            osb = opool.tile([128, 4 * E], F32, name=f"{pfx}osb{b}", tag="osb")
            for mt in range(4):
                do_proj_mt(yt, b, osb, mt)

        def load_x(b):
            xtb = xpool.tile([128, 2 * T], MM_DT, name=f"{pfx}xtb{b}", tag="xtb")
            for k in range(2):
                nc.sync.dma_start(xtb[:, k * T:(k + 1) * T], _mm(xt[b, k * 128:(k + 1) * 128, :]))
            return xtb

        # v_aug layout per s-chunk c: [st, 8 heads x 128] where per head
        # cols 0:64 = v+bias, cols 64:128 = ones (z-broadcast rows in yt mm).
        # ones blocks written once here; manual 2-deep rotation across batches.
        va_bufs = []
        for i in range(2):
            vab = vpool.tile([128, 4 * 1024], MM_DT, name=f"{pfx}vab{i}")
            va_bufs.append(vab)
            for c in range(4):
                va4 = vab[:, c * 1024:(c + 1) * 1024].rearrange("p (h m) -> p h m", h=H)
                nc.vector.memset(va4[:, :, 64:128], 1.0)

        xtb_next = load_x(0)
        prev = None  # (yt, b) awaiting projection
        for b in range(BL):
            xtb = xtb_next

            # ---- qkT(b): [1024(e), 500(t)] as [128, 8*500], e-tile j in cols ----
            qk = qkpool.tile([128, 8 * T], MM_DT, name=f"{pfx}qk{b}", tag="qk")
            for m in range(8):
                pq = ps_io.tile([128, T], F32, name=f"{pfx}pq{b}_{m}", tag="ps_io")
                if "qkv" in ABLATE:
                    continue
                for k in range(2):
                    nc.tensor.matmul(
                        pq[:],
                        _mm(wa[:, k * 1536 + m * 128:k * 1536 + (m + 1) * 128]),
                        _mm(xtb[:, k * T:(k + 1) * T]),
                        start=(k == 0), stop=(k == 1),
                    )
                if "qkcopy" in ABLATE:
                    continue
                # add per-partition bias (b_attn for q/k) while copying to SBUF
                on_dve = QKCOPY_DVE == "1" or (QKCOPY_DVE == "2" and m % 2 == 1)
                if on_dve:
                    nc.vector.tensor_scalar_add(qk[:, m * T:(m + 1) * T], pq[:],
                                                bqk_t[:, m:m + 1])
                else:
                    nc.scalar.activation(qk[:, m * T:(m + 1) * T], pq[:], AF.Identity,
                                         bias=bqk_t[:, m:m + 1])

            # ---- v(b): [500(t), 512(e)] + bias, into v_aug [128, 4*1024] ----
            va = va_bufs[b % 2]
            for mt in range(4):
                tt = TT[mt]
                pv = ps_io.tile([128, E], F32, name=f"{pfx}pv{b}_{mt}", tag="ps_io")
                if "va" in ABLATE:
                    continue
                for k in range(2):
                    nc.tensor.matmul(
                        pv[0:tt, :],
                        _mm(xtb[:, k * T + mt * 128:k * T + mt * 128 + tt]),
                        _mm(wa[:, k * 1536 + 1024:k * 1536 + 1536]),
                        start=(k == 0), stop=(k == 1),
                    )
                va3 = va[:, mt * 1024:(mt + 1) * 1024].rearrange("p (h m) -> p h m", h=H)
                nc.vector.tensor_add(
                    va3[0:tt, :, 0:64],
                    pv[0:tt, :].rearrange("p (h m) -> p h m", h=H),
                    bvb_t[0:tt, :].rearrange("p (h m) -> p h m", h=H),
                )

            if b + 1 < BL:
                xtb_next = load_x(b + 1)

            # ---- attention heads, z-chain pipelined 2 deep ----
            # (projection of batch b-1 is interleaved into hp=0 fillers)
            est = {}
            pyt = {}
            zr = {}
            zbs = {}

            def do_st_half(h0, h1, sp, pst):
                # ST[s,t] = k_h @ q_h^T for s-pair sp; exp(ST/8) -> est[h]
                for h in (h0, h1):
                    jq, oq = h // 2, (h % 2) * 64
                    jk, ok = 4 + h // 2, (h % 2) * 64
                    # [128, 2*512]: each 500-col matmul half bank-aligned
                    pst[h] = ps_st.tile([128, 1024], F32,
                                        name=f"{pfx}pst{b}_{h}_{sp}", tag="ps_st")
                    for si in range(2):
                        if "stmm" in ABLATE:
                            continue
                        s = 2 * sp + si
                        st = TT[s]
                        nc.tensor.matmul(
                            pst[h][0:st, si * 512:si * 512 + T],
                            _mm(qk[ok:ok + 64, jk * T + s * 128:jk * T + s * 128 + st]),
                            _mm(qk[oq:oq + 64, jq * T:(jq + 1) * T]),
                            start=True, stop=True,
                        )
                for h in (h0, h1):
                    if "exp" in ABLATE:
                        continue
                    pst3 = pst[h][:].rearrange("p (c m) -> p c m", c=2)
                    est3 = est[h][:, 2 * sp * T:(2 * sp + 2) * T].rearrange(
                        "p (c m) -> p c m", c=2)
                    nc.scalar.activation(est3[:, :, 0:T], pst3[:, :, 0:T],
                                         AF.Exp, scale=0.125)

            def do_yt(h):
                # yT[d,t] rows 0:64 (+ z bcast in rows 64:128) = [v_h|ones64]^T @ expST
                p = ps_yt.tile([128, T], F32, name=f"{pfx}pyt{b}_{h}", tag="ps_yt")
                pyt[h] = p
                e = est[h]
                if "yt" not in ABLATE:
                    for s in range(4):
                        st = TT[s]
                        nc.tensor.matmul(
                            p[:],
                            _mm(va[0:st, s * 1024 + 128 * h:s * 1024 + 128 * h + 128]),
                            _mm(e[0:st, s * T:(s + 1) * T]),
                            start=(s == 0), stop=(s == 3),
                        )
                if not NORM_DIV:
                    zs = zpool.tile([64, T], F32, name=f"{pfx}zbs{b}_{h}", tag="zbs")
                    zbs[h] = zs
                    if "z" not in ABLATE:
                        nc.vector.reciprocal(zs[:], p[64:128, :])

            def do_norm(h, yt):
                if "norm" in ABLATE:
                    return
                j, o = h // 2, (h % 2) * 64
                if NORM_DIV:
                    nc.vector.tensor_tensor(
                        yt[o:o + 64, j * T:(j + 1) * T], pyt[h][0:64, :],
                        pyt[h][64:128, :], mybir.AluOpType.divide,
                    )
                else:
                    nc.vector.tensor_mul(
                        yt[o:o + 64, j * T:(j + 1) * T], pyt[h][0:64, :], zbs[h][:]
                    )

            yt = ypool.tile([128, 4 * T], MM_DT, name=f"{pfx}yt{b}", tag="yt")
            osb_prev = None
            if prev is not None:
                osb_prev = opool.tile([128, 4 * E], F32,
                                      name=f"{pfx}osb{prev[1]}", tag="osb")
            for hp in range(H // 2):
                for h in (2 * hp, 2 * hp + 1):
                    est[h] = epool.tile([128, 4 * T], MM_DT,
                                        name=f"{pfx}est{b}_{h}", tag="est")
                pst = {}
                do_st_half(2 * hp, 2 * hp + 1, 0, pst)
                # filler A: PE/DVE work that is ready now, absorbing the
                # st(sp1)-waits-exp(sp0) PSUM-rotation stall
                if hp >= 1:
                    do_yt(2 * hp - 2)
                    do_norm(2 * hp - 2, yt)
                elif prev is not None:
                    do_proj_mt(prev[0], prev[1], osb_prev, 0)
                    do_proj_mt(prev[0], prev[1], osb_prev, 1)
                do_st_half(2 * hp, 2 * hp + 1, 1, pst)
                if hp >= 1:
                    do_yt(2 * hp - 1)
                    do_norm(2 * hp - 1, yt)
                elif prev is not None:
                    do_proj_mt(prev[0], prev[1], osb_prev, 2)
                    do_proj_mt(prev[0], prev[1], osb_prev, 3)
            for h in (H - 2, H - 1):
                do_yt(h)
                do_norm(h, yt)

            prev = (yt, b)
        do_proj(*prev)


_NC = None


def _get_nc():
    global _NC
    if _NC is None:
        _NC = build_nc()
    return _NC


def _np_bf16():
    import ml_dtypes
    return np.dtype(ml_dtypes.bfloat16)


def prep_inputs(x, w_attn, b_attn, w_proj, b_proj):
    x = np.asarray(x, np.float32)
    w_attn = np.asarray(w_attn, np.float32)
    b_attn = np.asarray(b_attn, np.float32)
    w_proj = np.asarray(w_proj, np.float32)
    b_proj = np.asarray(b_proj, np.float32)

    xt_all = np.ascontiguousarray(x.transpose(0, 2, 1))  # [B, C, T]
    wat = np.ascontiguousarray(w_attn.T)  # [C, 1536]
    wpt = np.ascontiguousarray(w_proj.T)  # [E, E]
    if MM_MODE == "bf16":
        bf = _np_bf16()
        xt_all = xt_all.astype(bf)
        wat = wat.astype(bf)
        wpt = wpt.astype(bf)
    bqk = np.ascontiguousarray(b_attn[:1024].reshape(8, 128).T)  # [128, 8]
    bvb = np.ascontiguousarray(np.tile(b_attn[1024:1536][None, :], (128, 1)))
    bpb = np.ascontiguousarray(np.tile(b_proj[None, :], (128, 1)))

    in_maps = []
    for c in range(N_CORES):
        in_maps.append({
            "xt": np.ascontiguousarray(xt_all[c * BL:(c + 1) * BL]),
            "wat": wat, "wpt": wpt, "bqk": bqk, "bvb": bvb, "bpb": bpb,
        })
    return in_maps


def kernel(x, w_attn, b_attn, w_proj, b_proj):
    nc = _get_nc()
    in_maps = prep_inputs(x, w_attn, b_attn, w_proj, b_proj)
    res = run_bass_kernel_spmd(nc, in_maps, core_ids=list(range(N_CORES)))
    out = np.concatenate([res.results[c]["out"] for c in range(N_CORES)], axis=0)
    return out.astype(np.float32)


# revision 23
# speedup vs baseline: 1.1130x; 1.0628x over previous
import sys

sys.path.insert(0, "/opt/trn_rl_repo")
import os as _os
import numpy as np
import concourse.bass as bass
import concourse.tile as tile
from concourse import bacc, mybir
from concourse.bass_utils import run_bass_kernel_spmd

F32 = mybir.dt.float32
F32R = mybir.dt.float32r
BF16 = mybir.dt.bfloat16
AF = mybir.ActivationFunctionType

B, T, C = 64, 500, 256
E, H, D = 512, 8, 64
N_CORES = 8
BL = B // N_CORES  # batches per core

# matmul operand mode: bf16 (1 cyc/row on hw), f32r (~2 cyc/row on hw), f32 (4)
MM_MODE = _os.environ.get("MM_MODE", "bf16")
# ablation for profiling: comma-set of {stmm, exp, yt, z, norm, qkv, va, proj}
ABLATE = set(filter(None, _os.environ.get("ABLATE", "").split(",")))

TT = [128, 128, 128, 116]  # t/s tile sizes (500 = 3*128 + 116)

MM_DT = {"bf16": BF16, "f32r": F32R, "f32": F32}[MM_MODE]
# dram x/w tensors are host-converted to bf16 in bf16 mode; else f32 (+bitcast)
IN_DT = BF16 if MM_MODE == "bf16" else F32

# tensor specs for timeit_hw's loop-variant builder: (name, shape, dtype)
INPUT_SPECS = [
    ("xt", (BL, C, T), IN_DT),
    ("wat", (C, 3 * E), IN_DT),
    ("wpt", (E, E), IN_DT),
    ("bqk", (128, 8), F32),
    ("bvb", (128, E), F32),
    ("bpb", (128, E), F32),
]
OUTPUT_SPEC = (BL, T, E)


def _mm(ap):
    # reinterpret f32 as f32r for the tensor engine (no-op in bf16/f32 modes)
    return ap if ap.dtype == MM_DT or MM_DT == BF16 else ap.bitcast(MM_DT)


def build_nc():
    nc = bacc.Bacc("TRN2", target_bir_lowering=False)
    aps = {}
    for name, shape, dt in INPUT_SPECS:
        aps[name] = nc.dram_tensor(name, list(shape), dt, kind="ExternalInput")
    out = nc.dram_tensor("out", list(OUTPUT_SPEC), F32, kind="ExternalOutput")

    with tile.TileContext(nc) as tc:
        _build_body(nc, tc, out=out, **aps)
    nc.compile()
    return nc


def _build_body(nc, tc, xt, wat, wpt, bqk, bvb, bpb, out, pfx=""):
    from contextlib import ExitStack

    ctx = ExitStack()
    with ctx:
        cpool = ctx.enter_context(tc.tile_pool(name="consts", bufs=1))
        xpool = ctx.enter_context(tc.tile_pool(name="x", bufs=2))
        qkpool = ctx.enter_context(tc.tile_pool(name="qk", bufs=2))
        vpool = ctx.enter_context(tc.tile_pool(name="v", bufs=2))
        epool = ctx.enter_context(tc.tile_pool(name="est", bufs=4))
        ypool = ctx.enter_context(tc.tile_pool(name="yt", bufs=2))
        opool = ctx.enter_context(tc.tile_pool(name="os", bufs=2))
        zpool = ctx.enter_context(tc.tile_pool(name="zr", bufs=3))
        # PSUM pools: 8 banks total
        EXPCHUNK = _os.environ.get("EXPCHUNK", "1") == "1"
        ps_io = ctx.enter_context(tc.tile_pool(name="ps_io", bufs=2, space="PSUM"))
        ps_st = ctx.enter_context(
            tc.tile_pool(name="ps_st", bufs=2 if EXPCHUNK else 4, space="PSUM"))
        ps_yt = ctx.enter_context(tc.tile_pool(name="ps_yt", bufs=2, space="PSUM"))

        # ---- constants ----
        wa = cpool.tile([128, 2 * 3 * E], MM_DT, name=f"{pfx}wa")  # 2 c-ktiles x [128,1536]
        for k in range(2):
            nc.sync.dma_start(wa[:, k * 1536:(k + 1) * 1536], _mm(wat[k * 128:(k + 1) * 128, :]))
        wp = cpool.tile([128, 4 * E], MM_DT, name=f"{pfx}wp")  # 4 e-ktiles x [128,512]
        for k in range(4):
            nc.sync.dma_start(wp[:, k * E:(k + 1) * E], _mm(wpt[k * 128:(k + 1) * 128, :]))
        bqk_t = cpool.tile([128, 8], F32, name=f"{pfx}bqk_t")
        nc.sync.dma_start(bqk_t[:], bqk[:, :])
        bvb_t = cpool.tile([128, E], F32, name=f"{pfx}bvb_t")
        nc.sync.dma_start(bvb_t[:], bvb[:, :])
        bpb_t = cpool.tile([128, E], F32, name=f"{pfx}bpb_t")
        nc.sync.dma_start(bpb_t[:], bpb[:, :])
        NORM_DIV = _os.environ.get("NORM_DIV", "0") == "1"
        QKCOPY_DVE = _os.environ.get("QKCOPY_DVE", "0")

        def do_proj_mt(yt, b, osb, mt):
            # out[t,f] = yT^T @ wpT + bproj (one 128-row t-tile)
            tt = TT[mt]
            po = ps_io.tile([128, E], F32, name=f"{pfx}po{b}_{mt}", tag="ps_io")
            if "proj" not in ABLATE:
                for k in range(4):
                    nc.tensor.matmul(
                        po[0:tt, :],
                        _mm(yt[:, k * T + mt * 128:k * T + mt * 128 + tt]),
                        _mm(wp[:, k * E:(k + 1) * E]),
                        start=(k == 0), stop=(k == 3),
                    )
                nc.vector.tensor_add(osb[0:tt, mt * E:(mt + 1) * E], po[0:tt, :], bpb_t[0:tt, :])
                nc.sync.dma_start(out[b, mt * 128:mt * 128 + tt, :], osb[0:tt, mt * E:(mt + 1) * E])

        def do_proj(yt, b):
            osb = opool.tile([128, 4 * E], F32, name=f"{pfx}osb{b}", tag="osb")
            for mt in range(4):
                do_proj_mt(yt, b, osb, mt)

        def load_x(b):
            xtb = xpool.tile([128, 2 * T], MM_DT, name=f"{pfx}xtb{b}", tag="xtb")
            for k in range(2):
                nc.sync.dma_start(xtb[:, k * T:(k + 1) * T], _mm(xt[b, k * 128:(k + 1) * 128, :]))
            return xtb

        # v_aug layout per s-chunk c: [st, 8 heads x 128] where per head
        # cols 0:64 = v+bias, cols 64:128 = ones (z-broadcast rows in yt mm).
        # ones blocks written once here; manual 2-deep rotation across batches.
        va_bufs = []
        for i in range(2):
            vab = vpool.tile([128, 4 * 1024], MM_DT, name=f"{pfx}vab{i}")
            va_bufs.append(vab)
            for c in range(4):
                va4 = vab[:, c * 1024:(c + 1) * 1024].rearrange("p (h m) -> p h m", h=H)
                nc.vector.memset(va4[:, :, 64:128], 1.0)

        xtb_next = load_x(0)
        prev = None  # (yt, b) awaiting projection
        for b in range(BL):
            xtb = xtb_next

            # ---- qkT(b): [1024(e), 500(t)] as [128, 8*500], e-tile j in cols ----
            qk = qkpool.tile([128, 8 * T], MM_DT, name=f"{pfx}qk{b}", tag="qk")
            for m in range(8):
                pq = ps_io.tile([128, T], F32, name=f"{pfx}pq{b}_{m}", tag="ps_io")
                if "qkv" in ABLATE:
                    continue
                for k in range(2):
                    nc.tensor.matmul(
                        pq[:],
                        _mm(wa[:, k * 1536 + m * 128:k * 1536 + (m + 1) * 128]),
                        _mm(xtb[:, k * T:(k + 1) * T]),
                        start=(k == 0), stop=(k == 1),
                    )
                if "qkcopy" in ABLATE:
                    continue
                # add per-partition bias (b_attn for q/k) while copying to SBUF
                on_dve = QKCOPY_DVE == "1" or (QKCOPY_DVE == "2" and m % 2 == 1)
                if on_dve:
                    nc.vector.tensor_scalar_add(qk[:, m * T:(m + 1) * T], pq[:],
                                                bqk_t[:, m:m + 1])
                else:
                    nc.scalar.activation(qk[:, m * T:(m + 1) * T], pq[:], AF.Identity,
                                         bias=bqk_t[:, m:m + 1])

            # ---- v(b): [500(t), 512(e)] + bias, into v_aug [128, 4*1024] ----
            va = va_bufs[b % 2]
            for mt in range(4):
                tt = TT[mt]
                pv = ps_io.tile([128, E], F32, name=f"{pfx}pv{b}_{mt}", tag="ps_io")
                if "va" in ABLATE:
                    continue
                for k in range(2):
                    nc.tensor.matmul(
                        pv[0:tt, :],
                        _mm(xtb[:, k * T + mt * 128:k * T + mt * 128 + tt]),
                        _mm(wa[:, k * 1536 + 1024:k * 1536 + 1536]),
                        start=(k == 0), stop=(k == 1),
                    )
                va3 = va[:, mt * 1024:(mt + 1) * 1024].rearrange("p (h m) -> p h m", h=H)
                nc.vector.tensor_add(
                    va3[0:tt, :, 0:64],
                    pv[0:tt, :].rearrange("p (h m) -> p h m", h=H),
                    bvb_t[0:tt, :].rearrange("p (h m) -> p h m", h=H),
                )

            if b + 1 < BL:
                xtb_next = load_x(b + 1)

            # ---- attention heads, z-chain pipelined 2 deep ----
            # (projection of batch b-1 is interleaved into hp=0 fillers)
            est = {}
            pyt = {}
            zr = {}
            zbs = {}

            def do_st_half(h0, h1, sp, pst):
                # ST[s,t] = k_h @ q_h^T for s-pair sp; exp(ST/8) -> est[h]
                for h in (h0, h1):
                    jq, oq = h // 2, (h % 2) * 64
                    jk, ok = 4 + h // 2, (h % 2) * 64
                    # [128, 2*512]: each 500-col matmul half bank-aligned
                    pst[h] = ps_st.tile([128, 1024], F32,
                                        name=f"{pfx}pst{b}_{h}_{sp}", tag="ps_st")
                    for si in range(2):
                        if "stmm" in ABLATE:
                            continue
                        s = 2 * sp + si
                        st = TT[s]
                        nc.tensor.matmul(
                            pst[h][0:st, si * 512:si * 512 + T],
                            _mm(qk[ok:ok + 64, jk * T + s * 128:jk * T + s * 128 + st]),
                            _mm(qk[oq:oq + 64, jq * T:(jq + 1) * T]),
                            start=True, stop=True,
                        )
                for h in (h0, h1):
                    if "exp" in ABLATE:
                        continue
                    # est is 512-padded per s-block: plain 2D exp over the full
                    # [128,1024] pst (cols 500:512 of each half are junk, unread)
                    nc.scalar.activation(est[h][:, sp * 1024:(sp + 1) * 1024],
                                         pst[h][:], AF.Exp, scale=0.125)

            def do_yt(h):
                # yT[d,t] rows 0:64 (+ z bcast in rows 64:128) = [v_h|ones64]^T @ expST
                p = ps_yt.tile([128, T], F32, name=f"{pfx}pyt{b}_{h}", tag="ps_yt")
                pyt[h] = p
                e = est[h]
                if "yt" not in ABLATE:
                    for s in range(4):
                        st = TT[s]
                        nc.tensor.matmul(
                            p[:],
                            _mm(va[0:st, s * 1024 + 128 * h:s * 1024 + 128 * h + 128]),
                            _mm(e[0:st, s * 512:s * 512 + T]),
                            start=(s == 0), stop=(s == 3),
                        )
                if not NORM_DIV:
                    zs = zpool.tile([64, T], F32, name=f"{pfx}zbs{b}_{h}", tag="zbs")
                    zbs[h] = zs
                    if "z" not in ABLATE:
                        nc.vector.reciprocal(zs[:], p[64:128, :])

            def do_norm(h, yt):
                if "norm" in ABLATE:
                    return
                j, o = h // 2, (h % 2) * 64
                if NORM_DIV:
                    nc.vector.tensor_tensor(
                        yt[o:o + 64, j * T:(j + 1) * T], pyt[h][0:64, :],
                        pyt[h][64:128, :], mybir.AluOpType.divide,
                    )
                else:
                    nc.vector.tensor_mul(
                        yt[o:o + 64, j * T:(j + 1) * T], pyt[h][0:64, :], zbs[h][:]
                    )

            yt = ypool.tile([128, 4 * T], MM_DT, name=f"{pfx}yt{b}", tag="yt")
            osb_prev = None
            if prev is not None:
                osb_prev = opool.tile([128, 4 * E], F32,
                                      name=f"{pfx}osb{prev[1]}", tag="osb")
            for hp in range(H // 2):
                for h in (2 * hp, 2 * hp + 1):
                    est[h] = epool.tile([128, 4 * 512], MM_DT,
                                        name=f"{pfx}est{b}_{h}", tag="est")
                pst = {}
                do_st_half(2 * hp, 2 * hp + 1, 0, pst)
                # filler A: PE/DVE work that is ready now, absorbing the
                # st(sp1)-waits-exp(sp0) PSUM-rotation stall
                if hp >= 1:
                    do_yt(2 * hp - 2)
                    do_norm(2 * hp - 2, yt)
                elif prev is not None:
                    do_proj_mt(prev[0], prev[1], osb_prev, 0)
                    do_proj_mt(prev[0], prev[1], osb_prev, 1)
                do_st_half(2 * hp, 2 * hp + 1, 1, pst)
                if hp >= 1:
                    do_yt(2 * hp - 1)
                    do_norm(2 * hp - 1, yt)
                elif prev is not None:
                    do_proj_mt(prev[0], prev[1], osb_prev, 2)
                    do_proj_mt(prev[0], prev[1], osb_prev, 3)
            for h in (H - 2, H - 1):
                do_yt(h)
                do_norm(h, yt)

            prev = (yt, b)
        do_proj(*prev)


_NC = None


def _get_nc():
    global _NC
    if _NC is None:
        _NC = build_nc()
    return _NC


def _np_bf16():
    import ml_dtypes
    return np.dtype(ml_dtypes.bfloat16)


def prep_inputs(x, w_attn, b_attn, w_proj, b_proj):
    x = np.asarray(x, np.float32)
    w_attn = np.asarray(w_attn, np.float32)
    b_attn = np.asarray(b_attn, np.float32)
    w_proj = np.asarray(w_proj, np.float32)
    b_proj = np.asarray(b_proj, np.float32)

    xt_all = np.ascontiguousarray(x.transpose(0, 2, 1))  # [B, C, T]
    wat = np.ascontiguousarray(w_attn.T)  # [C, 1536]
    wpt = np.ascontiguousarray(w_proj.T)  # [E, E]
    if MM_MODE == "bf16":
        bf = _np_bf16()
        xt_all = xt_all.astype(bf)
        wat = wat.astype(bf)
        wpt = wpt.astype(bf)
    bqk = np.ascontiguousarray(b_attn[:1024].reshape(8, 128).T)  # [128, 8]
    bvb = np.ascontiguousarray(np.tile(b_attn[1024:1536][None, :], (128, 1)))
    bpb = np.ascontiguousarray(np.tile(b_proj[None, :], (128, 1)))

    in_maps = []
    for c in range(N_CORES):
        in_maps.append({
            "xt": np.ascontiguousarray(xt_all[c * BL:(c + 1) * BL]),
            "wat": wat, "wpt": wpt, "bqk": bqk, "bvb": bvb, "bpb": bpb,
        })
    return in_maps


def kernel(x, w_attn, b_attn, w_proj, b_proj):
    nc = _get_nc()
    in_maps = prep_inputs(x, w_attn, b_attn, w_proj, b_proj)
    res = run_bass_kernel_spmd(nc, in_maps, core_ids=list(range(N_CORES)))
    out = np.concatenate([res.results[c]["out"] for c in range(N_CORES)], axis=0)
    return out.astype(np.float32)
